# revision 1
# baseline (speedup 1.0000x reference)
"""Trainium2 Bass kernel for nn_CustomFullyConnectedLayerGoogleTopK.

Reference computation:
    a = clip(K * softmax(alpha), 0, 1)                    # (4096,)
    W[rows, cols] += (V * a[:, None])  with rows=(j+i)%N, cols=j
    out = x @ W.T                                          # (256, 4096)

The scatter indices form a bijection (for each col j, row (j+i)%N hits every
row exactly once as i varies), so there is no actual accumulation:

    W[r, c] = V[(r - c) % N, c] * a[(r - c) % N]
    out[b, r] = sum_c x[b, c] * V[(r-c)%N, c] * a[(r-c)%N]

Sharding: output columns r are sharded 8 ways (512 per core) -> no collective;
each core reads only the diagonal band of V it needs (8 MB), all of x (4 MB),
and produces a disjoint out[:, r0:r0+512] slice.

Host-side prep is layout-only (static gather of V's wrapped diagonal band,
x transpose, row reversal, alpha roll + doubling); softmax, clipping, scaling
and the GEMM all run on device. The per-core r0 offset is absorbed into the
input layout (alpha is rolled by r0) so all 8 cores run one SPMD program.

Device-side layout trick: with the contraction rows presented in REVERSED
order (c = N-1-p for SBUF partition-row p), the skewed scale field the band
tiles need becomes the ascending Toeplitz  scale[p, j] = a2[1 + p + j]  where
a2 is `a` doubled. The (doubled, rolled) RAW alpha input is loaded directly in
this overlapping-window layout (one DMA per 4-block batch, partition step +1),
and the softmax scale is applied in place:

    piece = min(exp(alpha_piece + (ln K - ln sum_exp)), 1)
          = min(K * softmax(alpha), 1)     elementwise on the piece

using the otherwise-idle Scalar engine for the biased Exp and GpSimd for the
clip, so nothing round-trips through DRAM and the scale tiles are ready a few
microseconds into the kernel.

The GEMM runs in float32r (full-rate PE mode, ~1.5e-4 rms rel error measured
on HW vs fp64), accumulating fp32 in PSUM over all 32 contraction blocks. The
xT input is declared float32r directly (same 4-byte layout; the PE rounds
internally) so it loads as one plain 4 MB HWDGE DMA with no cast pass.
"""

import math
import os
import sys

import numpy as np

for _p in ("/opt/trn_rl_repo", "/root/.axon_site/_ro/trn_rl_repo"):
    if os.path.isdir(_p) and _p not in sys.path:
        sys.path.append(_p)

import concourse.bacc as bacc
import concourse.bass as bass
import concourse.mybir as mybir
import concourse.tile as tile
from concourse.bass_utils import run_bass_kernel_spmd

F32 = mybir.dt.float32
F32R = mybir.dt.float32r

N = 4096          # IN_F == OUT_F == N_PERM == DIAG
B = 256           # batch
NCORES = 8
RW = N // NCORES  # 512 output columns per core
K_TOPK = 3687     # ceil(0.9 * 4096 * 4096 / 4096)
CB = 128          # contraction block (SBUF partition count)
NCB = N // CB     # 32 contraction blocks
TB = 4            # contraction blocks per DMA/multiply batch
NBATCH = NCB // TB
GPW = RW + (TB - 1) * CB  # 896: width of one Toeplitz scale piece


def _strided_cols(ap2d, col_off, t_step, n_t, inner):
    """[128, W] SBUF tile -> [128, n_t, inner] view starting at col_off with
    column stride t_step between t-slices (overlap allowed)."""
    pstep = ap2d.ap[0][0]
    return bass.AP(
        ap2d.tensor, ap2d.offset + col_off,
        [[pstep, 128], [t_step, n_t], [1, inner]],
    )


def _build_program():
    nc = bacc.Bacc("TRN2", target_bir_lowering=False, debug=False)

    band = nc.dram_tensor("band", [N, RW], F32, kind="ExternalInput").ap()
    xT = nc.dram_tensor("xT", [N, B], F32R, kind="ExternalInput").ap()
    alpha2 = nc.dram_tensor("alpha2", [2 * N], F32, kind="ExternalInput").ap()
    out = nc.dram_tensor("out", [B, RW], F32, kind="ExternalOutput").ap()

    with tile.TileContext(nc) as tc:
        with (
            tc.tile_pool(name="small", bufs=1) as sp,
            tc.tile_pool(name="gpool", bufs=1) as gp,
            tc.tile_pool(name="vb", bufs=6) as vbp,
            tc.tile_pool(name="wt", bufs=4) as wtp,
            tc.tile_pool(name="xtp", bufs=1) as xtp,
            tc.tile_pool(name="opool", bufs=2) as op,
            tc.tile_pool(name="psum", bufs=1, space="PSUM") as pp,
            tc.tile_pool(name="psum_s", bufs=1, space="PSUM") as pps,
        ):
            # ---- softmax normalizer: bias = ln K - ln sum(exp(alpha)) ----
            alpha_sb = sp.tile([128, N // 128], F32)
            nc.gpsimd.dma_start(
                alpha_sb[:], alpha2[0:N].rearrange("(p f) -> p f", p=128)
            )
            # alpha pieces in overlapping Toeplitz layout (no deps: start now)
            ag = []
            for q in range(NBATCH):
                agq = gp.tile([128, GPW], F32, tag=f"g{q}")
                src = bass.AP(
                    alpha2.tensor, alpha2.offset + 1 + q * TB * CB,
                    [[1, 128], [1, GPW]],
                )
                nc.gpsimd.dma_start(agq[:], src)
                ag.append(agq)

            exp_sb = sp.tile([128, N // 128], F32)
            rowsum = sp.tile([128, 1], F32)
            # alpha is uniform in [0,1): no max-subtraction needed for stability
            nc.scalar.activation(
                exp_sb[:], alpha_sb[:], mybir.ActivationFunctionType.Exp,
                accum_out=rowsum[:],
            )
            ones = sp.tile([128, 128], F32)
            nc.vector.memset(ones[:], 1.0)
            tot_ps = pps.tile([128, 1], F32)
            # total = ones.T @ rowsum -> per-partition copy of the full sum
            nc.tensor.matmul(tot_ps[:], ones[:], rowsum[:], start=True, stop=True)
            ln_sum = sp.tile([128, 1], F32)
            nc.scalar.activation(
                ln_sum[:], tot_ps[:], mybir.ActivationFunctionType.Ln
            )
            # bias_neg = ln(sum) - ln(K);  min(K*softmax, 1) = exp(min(z, 0))
            # with z = alpha - bias_neg, and min(z,0) = -relu(-z), so two ACT
            # passes per piece: relu(-alpha + bias_neg) then exp(-that).
            bias_neg = sp.tile([128, 1], F32)
            nc.vector.tensor_scalar_add(
                bias_neg[:], ln_sum[:], -float(math.log(K_TOPK))
            )
            relu_pool = []
            for i in range(2):
                rt_i = sp.tile([128, GPW], F32, tag=f"relu{i}", name=f"relu{i}")
                relu_pool.append(rt_i)
            for q in range(NBATCH):
                rt = relu_pool[q % 2]
                nc.scalar.activation(
                    rt[:], ag[q][:], mybir.ActivationFunctionType.Relu,
                    bias=bias_neg[:, 0:1], scale=-1.0,
                )
                nc.scalar.activation(
                    ag[q][:], rt[:], mybir.ActivationFunctionType.Exp,
                    scale=-1.0,
                )

            # ---- whole xT in one DMA (f32r, plain HWDGE) ----
            xt = xtp.tile([128, NCB, B], F32R)
            nc.scalar.dma_start(
                xt[:], xT.rearrange("(n p) b -> p n b", p=128)
            )

            # ---- main loop: batches of (band * scale) -> matmul pairs ----
            psum0 = pp.tile([128, RW], F32)
            psum1 = pp.tile([128, RW], F32)
            for qi in range(NBATCH):
                q0 = qi * TB
                rows = slice(q0 * CB, (q0 + TB) * CB)
                vb = vbp.tile([128, TB, RW], F32)
                eng = nc.sync if qi < 6 else nc.scalar
                eng.dma_start(
                    vb[:], band[rows, :].rearrange("(t p) j -> p t j", p=128)
                )
                wt = wtp.tile([128, TB, RW], F32R)
                nc.vector.tensor_tensor(
                    wt[:], vb[:], _strided_cols(ag[qi], 0, CB, TB, RW),
                    mybir.AluOpType.mult,
                )
                for t in range(TB):
                    k = q0 + t
                    nc.tensor.matmul(psum0[:], xt[:, k, 0:128], wt[:, t, :],
                                     start=(k == 0), stop=(k == NCB - 1))
                    nc.tensor.matmul(psum1[:], xt[:, k, 128:256], wt[:, t, :],
                                     start=(k == 0), stop=(k == NCB - 1))

            # ---- PSUM -> SBUF -> DRAM ----
            o0 = op.tile([128, RW], F32)
            nc.vector.tensor_copy(o0[:], psum0[:])
            nc.scalar.dma_start(out[0:128, :], o0[:])
            o1 = op.tile([128, RW], F32)
            nc.vector.tensor_copy(o1[:], psum1[:])
            nc.scalar.dma_start(out[128:256, :], o1[:])

    nc.compile()
    return nc


_NC_CACHE = []


def _get_program():
    if not _NC_CACHE:
        _NC_CACHE.append(_build_program())
    return _NC_CACHE[0]


def prepare_in_maps(x: np.ndarray, V: np.ndarray, alpha: np.ndarray):
    """Layout-only sharding of the full inputs into 8 per-core input maps."""
    x = np.ascontiguousarray(np.asarray(x, dtype=np.float32))
    V = np.ascontiguousarray(np.asarray(V, dtype=np.float32))
    alpha = np.ascontiguousarray(np.asarray(alpha, dtype=np.float32))

    # rows presented in reversed order (c = N-1-p); see module docstring
    xT = np.ascontiguousarray(x.T[::-1, :])  # (N, B)

    # VtD[c, t] = V[t % N, c] for t in [0, 2N): doubled transpose for wrap-free
    # band extraction. band_m[c, j] = V[(r0 + j - c) % N, c]
    #              = VtD[c, N + r0 + j - c]
    Vt = np.ascontiguousarray(V.T)
    VtD = np.concatenate([Vt, Vt], axis=1)  # (N, 2N)
    flat = VtD.reshape(-1)
    isz = flat.itemsize

    in_maps = []
    for m in range(NCORES):
        r0 = m * RW
        start = N + r0  # element offset of band_m[0, 0] in flat
        band_m = np.lib.stride_tricks.as_strided(
            flat[start:], shape=(N, RW), strides=((2 * N - 1) * isz, isz),
        )
        am = np.roll(alpha, -r0)
        in_maps.append({
            "band": np.ascontiguousarray(band_m[::-1, :]),
            "xT": xT,
            "alpha2": np.ascontiguousarray(np.concatenate([am, am])),
        })
    return in_maps


def gather_output(results) -> np.ndarray:
    return np.concatenate([results[m]["out"] for m in range(NCORES)], axis=1)


def kernel(x: np.ndarray, V: np.ndarray, alpha: np.ndarray) -> np.ndarray:
    in_maps = prepare_in_maps(x, V, alpha)
    nc = _get_program()
    res = run_bass_kernel_spmd(nc, in_maps, core_ids=list(range(NCORES)))
    return gather_output(res.results)



# revision 2
# speedup vs baseline: 1.6184x; 1.6184x over previous
"""Trainium2 Bass kernel for nn_CustomFullyConnectedLayerGoogleTopK.

Reference computation:
    a = clip(K * softmax(alpha), 0, 1)                    # (4096,)
    W[rows, cols] += (V * a[:, None])  with rows=(j+i)%N, cols=j
    out = x @ W.T                                          # (256, 4096)

The scatter indices form a bijection (for each col j, row (j+i)%N hits every
row exactly once as i varies), so there is no actual accumulation:

    W[r, c] = V[(r - c) % N, c] * a[(r - c) % N]
    out[b, r] = sum_c x[b, c] * V[(r-c)%N, c] * a[(r-c)%N]

Sharding: output columns r are sharded 8 ways (512 per core) -> no collective;
each core reads only the diagonal band of V it needs, all of x, and produces a
disjoint out[:, r0:r0+512] slice.

The kernel is memory-bound, so the band and xT inputs are fed to the device as
bfloat16 (host-side cast; measured end-to-end max rel err ~3e-3 vs the fp32
reference, comfortably inside the 2e-2 gate), halving HBM traffic to ~7.6 MB
per core. PSUM accumulation stays fp32 across the full 4096-deep contraction.

Device-side layout trick: with the contraction rows presented in REVERSED
order (c = N-1-p for SBUF partition-row p), the skewed scale field the band
tiles need becomes the ascending Toeplitz  scale[p, j] = a2[1 + p + j]  where
a2 is the rolled-by-r0, doubled raw alpha.  One compact strip
agf[128, 4480] = a2[1 + p + u]  (u in [0, 4480)) serves every contraction
block k via the column-offset view agf[:, 128k : 128k+512], so the amplified
alpha read is 1.1 MB (bf16) instead of 8 separate 896-wide windows.

The softmax scale is reformulated as  s = min((K / sum(exp(alpha))) * exp(a2), 1)
so the strip's Exp pass has no bias dependency (it starts as soon as the
window DMA lands, and no Ln activation table is ever loaded); the runtime
scalar cK = K/sum is folded into the per-slice DVE pass (mult+min in one
tensor_scalar).

DMA queue assignment keeps the band stream unobstructed: Sync issues the 8
band chunks, Scalar issues the 4 xT chunks up front (before any ACTIVATE can
clog its FIFO) and the 2 output stores at the end, GpSimd (SWDGE) carries the
alpha strip + compact alpha. Band/xT are pre-tiled on the host to
partition-major [128, k, :] so every DMA descriptor is a 4 KB contiguous run.
"""

import math
import os
import sys

import numpy as np

for _p in ("/opt/trn_rl_repo", "/root/.axon_site/_ro/trn_rl_repo"):
    if os.path.isdir(_p) and _p not in sys.path:
        sys.path.append(_p)

import ml_dtypes

import concourse.bacc as bacc
import concourse.bass as bass
import concourse.mybir as mybir
import concourse.tile as tile
from concourse.bass_utils import run_bass_kernel_spmd

F32 = mybir.dt.float32
BF16 = mybir.dt.bfloat16
NP_BF16 = ml_dtypes.bfloat16

N = 4096          # IN_F == OUT_F == N_PERM == DIAG
B = 256           # batch
NCORES = 8
RW = N // NCORES  # 512 output columns per core
K_TOPK = 3687     # ceil(0.9 * 4096 * 4096 / 4096)
CB = 128          # contraction block (SBUF partition count)
NCB = N // CB     # 32 contraction blocks
TB = 4            # contraction blocks per band-chunk / wt-multiply batch
NBATCH = NCB // TB  # 8
XCH = 8           # contraction blocks per xT DMA chunk
NXCH = NCB // XCH   # 4
WCOLS = (NCB - 1) * CB + RW  # 4480: width of the Toeplitz scale strip
WCH = 2240        # strip DMA chunk width (2 chunks)
SLICE = 512       # scale-processing slice width (9 slices: 8x512 + 1x384)


def _strided_cols(ap2d, col_off, t_step, n_t, inner):
    """[128, W] SBUF tile -> [128, n_t, inner] view starting at col_off with
    column stride t_step between t-slices (overlap allowed)."""
    pstep = ap2d.ap[0][0]
    return bass.AP(
        ap2d.tensor, ap2d.offset + col_off,
        [[pstep, 128], [t_step, n_t], [1, inner]],
    )


def _build_program():
    nc = bacc.Bacc("TRN2", target_bir_lowering=False, debug=False)

    band_t = nc.dram_tensor("band_t", [128, NCB, RW], BF16, kind="ExternalInput").ap()
    xt_t = nc.dram_tensor("xt_t", [128, NCB, B], BF16, kind="ExternalInput").ap()
    al32 = nc.dram_tensor("al32", [N], F32, kind="ExternalInput").ap()
    a2bf = nc.dram_tensor("a2bf", [2 * N], BF16, kind="ExternalInput").ap()
    out = nc.dram_tensor("out", [B, RW], F32, kind="ExternalOutput").ap()

    with tile.TileContext(nc) as tc:
        with (
            tc.tile_pool(name="small", bufs=1) as sp,
            tc.tile_pool(name="agp", bufs=1) as agp,
            tc.tile_pool(name="bsb", bufs=1) as bsbp,
            tc.tile_pool(name="etmp", bufs=2) as etp,
            tc.tile_pool(name="wt", bufs=4) as wtp,
            tc.tile_pool(name="xtp", bufs=1) as xtp,
            tc.tile_pool(name="opool", bufs=2) as op,
            tc.tile_pool(name="psum", bufs=1, space="PSUM") as pp,
            tc.tile_pool(name="psum_s", bufs=1, space="PSUM") as pps,
        ):
            # ---- all input DMAs issued first, consumption order per ring ----
            bsb = bsbp.tile([128, NCB, RW], BF16)
            for c in range(NBATCH):
                ks = slice(c * TB, (c + 1) * TB)
                nc.sync.dma_start(bsb[:, ks, :], band_t[:, ks, :])

            xt = xtp.tile([128, NCB, B], BF16)
            for c in range(NXCH):
                ks = slice(c * XCH, (c + 1) * XCH)
                nc.scalar.dma_start(xt[:, ks, :], xt_t[:, ks, :])

            alpha_sb = sp.tile([128, N // 128], F32)
            nc.gpsimd.dma_start(
                alpha_sb[:], al32.rearrange("(p f) -> p f", p=128)
            )
            agf = agp.tile([128, WCOLS], BF16)
            for wc in range(WCOLS // WCH):
                src = bass.AP(
                    a2bf.tensor, a2bf.offset + 1 + wc * WCH,
                    [[1, 128], [1, WCH]],
                )
                nc.gpsimd.dma_start(agf[:, wc * WCH:(wc + 1) * WCH], src)

            # ---- softmax normalizer: cK = K / sum(exp(alpha)) ----
            exp_sb = sp.tile([128, N // 128], F32)
            rowsum = sp.tile([128, 1], F32)
            # alpha is uniform in [0,1): no max-subtraction needed for stability
            nc.scalar.activation(
                exp_sb[:], alpha_sb[:], mybir.ActivationFunctionType.Exp,
                accum_out=rowsum[:],
            )
            ones = sp.tile([128, 128], F32)
            nc.vector.memset(ones[:], 1.0)
            tot_ps = pps.tile([128, 1], F32)
            # total = ones.T @ rowsum -> per-partition copy of the full sum
            nc.tensor.matmul(tot_ps[:], ones[:], rowsum[:], start=True, stop=True)
            inv_s = sp.tile([128, 1], F32)
            nc.vector.reciprocal(inv_s[:], tot_ps[:])
            cK = sp.tile([128, 1], F32)
            nc.vector.tensor_scalar_mul(cK[:], inv_s[:], float(K_TOPK))

            # ---- scale strip: s = min(cK * exp(a2), 1), slice by slice ----
            n_slices = (WCOLS + SLICE - 1) // SLICE
            for s in range(n_slices):
                cols = slice(s * SLICE, min((s + 1) * SLICE, WCOLS))
                w = cols.stop - cols.start
                et = etp.tile([128, SLICE], BF16, tag=f"et{s % 2}")
                nc.scalar.activation(
                    et[:, 0:w], agf[:, cols], mybir.ActivationFunctionType.Exp
                )
                nc.vector.tensor_scalar(
                    agf[:, cols], et[:, 0:w], cK[:, 0:1], 1.0,
                    mybir.AluOpType.mult, mybir.AluOpType.min,
                )

            # ---- main loop: wt = band * scale -> matmul pairs ----
            psum0 = pp.tile([128, RW], F32)
            psum1 = pp.tile([128, RW], F32)
            for q in range(NBATCH):
                wt = wtp.tile([128, TB, RW], BF16)
                nc.vector.tensor_tensor(
                    wt[:], bsb[:, q * TB:(q + 1) * TB, :],
                    _strided_cols(agf, q * TB * CB, CB, TB, RW),
                    mybir.AluOpType.mult,
                )
                for t in range(TB):
                    k = q * TB + t
                    nc.tensor.matmul(psum0[:], xt[:, k, 0:128], wt[:, t, :],
                                     start=(k == 0), stop=(k == NCB - 1))
                    nc.tensor.matmul(psum1[:], xt[:, k, 128:256], wt[:, t, :],
                                     start=(k == 0), stop=(k == NCB - 1))

            # ---- PSUM -> SBUF -> DRAM ----
            o0 = op.tile([128, RW], F32)
            nc.vector.tensor_copy(o0[:], psum0[:])
            nc.scalar.dma_start(out[0:128, :], o0[:])
            o1 = op.tile([128, RW], F32)
            nc.vector.tensor_copy(o1[:], psum1[:])
            nc.scalar.dma_start(out[128:256, :], o1[:])

    nc.compile()
    return nc


_NC_CACHE = []


def _get_program():
    if not _NC_CACHE:
        _NC_CACHE.append(_build_program())
    return _NC_CACHE[0]


def prepare_in_maps(x: np.ndarray, V: np.ndarray, alpha: np.ndarray):
    """Shard + lay out the full inputs into 8 per-core input maps (bf16)."""
    x = np.ascontiguousarray(np.asarray(x, dtype=np.float32))
    V = np.ascontiguousarray(np.asarray(V, dtype=np.float32))
    alpha = np.ascontiguousarray(np.asarray(alpha, dtype=np.float32))

    # rows presented in reversed order (c = N-1-p); see module docstring
    xT_rev = np.ascontiguousarray(x.T[::-1, :]).astype(NP_BF16)  # (N, B)
    xt_t = np.ascontiguousarray(
        xT_rev.reshape(NCB, 128, B).transpose(1, 0, 2)
    )  # [p, k, b]

    # VtD[c, t] = V[t % N, c] for t in [0, 2N): doubled transpose for wrap-free
    # band extraction. band_m[c, j] = V[(r0 + j - c) % N, c] = VtD[c, N+r0+j-c]
    Vt = np.ascontiguousarray(V.T).astype(NP_BF16)
    VtD = np.concatenate([Vt, Vt], axis=1)  # (N, 2N) bf16
    flat = VtD.reshape(-1)
    isz = flat.itemsize

    in_maps = []
    for m in range(NCORES):
        r0 = m * RW
        start = N + r0  # element offset of band_m[0, 0] in flat
        band_m = np.lib.stride_tricks.as_strided(
            flat[start:], shape=(N, RW), strides=((2 * N - 1) * isz, isz),
        )
        band_rev = np.ascontiguousarray(band_m[::-1, :])  # (N, RW) bf16
        band_t = np.ascontiguousarray(
            band_rev.reshape(NCB, 128, RW).transpose(1, 0, 2)
        )  # [p, k, j]
        am = np.roll(alpha, -r0)
        in_maps.append({
            "band_t": band_t,
            "xt_t": xt_t,
            "al32": alpha,
            "a2bf": np.concatenate([am, am]).astype(NP_BF16),
        })
    return in_maps


def gather_output(results) -> np.ndarray:
    return np.concatenate([results[m]["out"] for m in range(NCORES)], axis=1)


def kernel(x: np.ndarray, V: np.ndarray, alpha: np.ndarray) -> np.ndarray:
    in_maps = prepare_in_maps(x, V, alpha)
    nc = _get_program()
    res = run_bass_kernel_spmd(nc, in_maps, core_ids=list(range(NCORES)))
    return gather_output(res.results)


# revision 3
# speedup vs baseline: 1.6272x; 1.0054x over previous
"""Trainium2 Bass kernel for nn_CustomFullyConnectedLayerGoogleTopK.

Reference computation:
    a = clip(K * softmax(alpha), 0, 1)                    # (4096,)
    W[rows, cols] += (V * a[:, None])  with rows=(j+i)%N, cols=j
    out = x @ W.T                                          # (256, 4096)

The scatter indices form a bijection (for each col j, row (j+i)%N hits every
row exactly once as i varies), so there is no actual accumulation:

    W[r, c] = V[(r - c) % N, c] * a[(r - c) % N]
    out[b, r] = sum_c x[b, c] * V[(r-c)%N, c] * a[(r-c)%N]

Sharding: output columns r are sharded 8 ways (512 per core) -> no collective;
each core reads only the diagonal band of V it needs, all of x, and produces a
disjoint out[:, r0:r0+512] slice.

The kernel is memory-bound, so the band and xT inputs are fed to the device as
bfloat16 (host-side cast; measured end-to-end max rel err ~3e-3 vs the fp32
reference, inside the 2e-2 gate), halving HBM traffic to ~7.6 MB per core.
PSUM accumulation stays fp32 across the full 4096-deep contraction.

Device-side layout trick: with the contraction rows presented in REVERSED
order (c = N-1-p for SBUF partition-row p), the skewed scale field the band
tiles need becomes the ascending Toeplitz  scale[p, j] = a2[1 + p + j]  where
a2 is the rolled-by-r0, doubled raw alpha.  One compact strip
agf[128, 4480] = a2[1 + p + u]  serves every contraction block k via the
column-offset view agf[:, 128k : 128k+512].

Scale algebra:  clip(K*softmax(alpha), 0, 1) = cK * min(exp(alpha), invK)
with invK = sum(exp(alpha))/K and cK = K/sum(exp(alpha)).  So the pipeline is
  - ACT: in-place Exp over the strip (no bias -> starts as soon as the
    window DMA lands; no Ln table ever loads),
  - DVE: one fused scalar_tensor_tensor per band batch,
        wt = (E min invK) * band        (min + multiply in a single pass),
  - the cK factor rides the PSUM eviction (tensor_scalar mult).

DMA layout: everything big goes over the two HWDGE rings.  Sync carries the
scale-strip window chunks interleaved with the 9 band chunks in consumption
order; Scalar carries the 4 xT chunks up front and one output store at the
end (the other store goes on Sync).  GpSimd/SWDGE only carries the tiny
compact-alpha read.  Band/xT are pre-tiled on the host to partition-major
[128, k, :] so every DMA descriptor is a >=2 KB contiguous run.  The last two
band chunks are 2 blocks instead of 4 to shorten the post-last-byte tail, and
outputs are stored as bf16 (upcast on host) to halve the tail store.
"""

import math
import os
import sys

import numpy as np

for _p in ("/opt/trn_rl_repo", "/root/.axon_site/_ro/trn_rl_repo"):
    if os.path.isdir(_p) and _p not in sys.path:
        sys.path.append(_p)

import ml_dtypes

import concourse.bacc as bacc
import concourse.bass as bass
import concourse.mybir as mybir
import concourse.tile as tile
from concourse.bass_utils import run_bass_kernel_spmd

F32 = mybir.dt.float32
BF16 = mybir.dt.bfloat16
NP_BF16 = ml_dtypes.bfloat16

N = 4096          # IN_F == OUT_F == N_PERM == DIAG
B = 256           # batch
NCORES = 8
RW = N // NCORES  # 512 output columns per core
K_TOPK = 3687     # ceil(0.9 * 4096 * 4096 / 4096)
CB = 128          # contraction block (SBUF partition count)
NCB = N // CB     # 32 contraction blocks
XCH = 8           # contraction blocks per xT DMA chunk
NXCH = NCB // XCH   # 4
WCOLS = (NCB - 1) * CB + RW  # 4480: width of the Toeplitz scale strip

# band chunk boundaries (in blocks): small tail chunks
BCHUNKS = [(0, 4), (4, 8), (8, 12), (12, 16), (16, 20), (20, 24), (24, 28),
           (28, 30), (30, 32)]
# window chunk boundaries (in strip columns)
WCHUNKS = [(0, 1408), (1408, 2944), (2944, 4480)]
# sync-ring issue order: window chunks just-in-time between band chunks
SYNC_ORDER = [("w", 0), ("b", 0), ("b", 1), ("w", 1), ("b", 2), ("b", 3),
              ("b", 4), ("w", 2), ("b", 5), ("b", 6), ("b", 7), ("b", 8)]


def _strided_cols(ap2d, col_off, t_step, n_t, inner):
    """[128, W] SBUF tile -> [128, n_t, inner] view starting at col_off with
    column stride t_step between t-slices (overlap allowed)."""
    pstep = ap2d.ap[0][0]
    return bass.AP(
        ap2d.tensor, ap2d.offset + col_off,
        [[pstep, 128], [t_step, n_t], [1, inner]],
    )


def _build_program():
    nc = bacc.Bacc("TRN2", target_bir_lowering=False, debug=False)

    band_t = nc.dram_tensor("band_t", [128, NCB, RW], BF16, kind="ExternalInput").ap()
    xt_t = nc.dram_tensor("xt_t", [128, NCB, B], BF16, kind="ExternalInput").ap()
    al32 = nc.dram_tensor("al32", [N], F32, kind="ExternalInput").ap()
    a2bf = nc.dram_tensor("a2bf", [2 * N], BF16, kind="ExternalInput").ap()
    out = nc.dram_tensor("out", [B, RW], BF16, kind="ExternalOutput").ap()

    with tile.TileContext(nc) as tc:
        with (
            tc.tile_pool(name="small", bufs=1) as sp,
            tc.tile_pool(name="agp", bufs=1) as agp,
            tc.tile_pool(name="bsb", bufs=1) as bsbp,
            tc.tile_pool(name="wt", bufs=4) as wtp,
            tc.tile_pool(name="xtp", bufs=1) as xtp,
            tc.tile_pool(name="opool", bufs=2) as op,
            tc.tile_pool(name="psum", bufs=1, space="PSUM") as pp,
            tc.tile_pool(name="psum_s", bufs=1, space="PSUM") as pps,
        ):
            # ---- all input DMAs issued first, consumption order per ring ----
            agf = agp.tile([128, WCOLS], BF16)
            bsb = bsbp.tile([128, NCB, RW], BF16)
            for kind, i in SYNC_ORDER:
                if kind == "w":
                    c0, c1 = WCHUNKS[i]
                    src = bass.AP(
                        a2bf.tensor, a2bf.offset + 1 + c0,
                        [[1, 128], [1, c1 - c0]],
                    )
                    nc.sync.dma_start(agf[:, c0:c1], src)
                else:
                    k0, k1 = BCHUNKS[i]
                    nc.sync.dma_start(bsb[:, k0:k1, :], band_t[:, k0:k1, :])

            xt = xtp.tile([128, NCB, B], BF16)
            for c in range(NXCH):
                ks = slice(c * XCH, (c + 1) * XCH)
                nc.scalar.dma_start(xt[:, ks, :], xt_t[:, ks, :])

            alpha_sb = sp.tile([128, N // 128], F32)
            nc.gpsimd.dma_start(
                alpha_sb[:], al32.rearrange("(p f) -> p f", p=128)
            )

            # ---- normalizer: invK = sum(exp(alpha))/K, cK = K/sum ----
            exp_sb = sp.tile([128, N // 128], F32)
            rowsum = sp.tile([128, 1], F32)
            # alpha is uniform in [0,1): no max-subtraction needed for stability
            nc.scalar.activation(
                exp_sb[:], alpha_sb[:], mybir.ActivationFunctionType.Exp,
                accum_out=rowsum[:],
            )
            ones = sp.tile([128, 128], F32)
            nc.vector.memset(ones[:], 1.0)
            tot_ps = pps.tile([128, 1], F32)
            # total = ones.T @ rowsum -> per-partition copy of the full sum
            nc.tensor.matmul(tot_ps[:], ones[:], rowsum[:], start=True, stop=True)
            invK = sp.tile([128, 1], F32)
            nc.vector.tensor_scalar_mul(invK[:], tot_ps[:], 1.0 / float(K_TOPK))
            inv_s = sp.tile([128, 1], F32)
            nc.vector.reciprocal(inv_s[:], tot_ps[:])
            cK = sp.tile([128, 1], F32)
            nc.vector.tensor_scalar_mul(cK[:], inv_s[:], float(K_TOPK))

            # ---- strip: E = exp(a2) in place, one ACT pass per window chunk
            for c0, c1 in WCHUNKS:
                nc.scalar.activation(
                    agf[:, c0:c1], agf[:, c0:c1],
                    mybir.ActivationFunctionType.Exp,
                )

            # ---- main loop: wt = (E min invK) * band -> matmul pairs ----
            psum0 = pp.tile([128, RW], F32)
            psum1 = pp.tile([128, RW], F32)
            for q, (k0, k1) in enumerate(BCHUNKS):
                tb = k1 - k0
                wt = wtp.tile([128, 4, RW], BF16, tag=f"wt{q % 4}")
                nc.vector.scalar_tensor_tensor(
                    wt[:, 0:tb, :],
                    _strided_cols(agf, k0 * CB, CB, tb, RW),
                    invK[:, 0:1],
                    bsb[:, k0:k1, :],
                    mybir.AluOpType.min,
                    mybir.AluOpType.mult,
                )
                for t in range(tb):
                    k = k0 + t
                    nc.tensor.matmul(psum0[:], xt[:, k, 0:128], wt[:, t, :],
                                     start=(k == 0), stop=(k == NCB - 1))
                    nc.tensor.matmul(psum1[:], xt[:, k, 128:256], wt[:, t, :],
                                     start=(k == 0), stop=(k == NCB - 1))

            # ---- PSUM -> (x cK) -> SBUF bf16 -> DRAM, one store per ring ----
            o0 = op.tile([128, RW], BF16)
            nc.vector.tensor_scalar_mul(o0[:], psum0[:], cK[:, 0:1])
            nc.sync.dma_start(out[0:128, :], o0[:])
            o1 = op.tile([128, RW], BF16)
            nc.vector.tensor_scalar_mul(o1[:], psum1[:], cK[:, 0:1])
            nc.scalar.dma_start(out[128:256, :], o1[:])

    nc.compile()
    return nc


_NC_CACHE = []


def _get_program():
    if not _NC_CACHE:
        _NC_CACHE.append(_build_program())
    return _NC_CACHE[0]


def prepare_in_maps(x: np.ndarray, V: np.ndarray, alpha: np.ndarray):
    """Shard + lay out the full inputs into 8 per-core input maps (bf16)."""
    x = np.ascontiguousarray(np.asarray(x, dtype=np.float32))
    V = np.ascontiguousarray(np.asarray(V, dtype=np.float32))
    alpha = np.ascontiguousarray(np.asarray(alpha, dtype=np.float32))

    # rows presented in reversed order (c = N-1-p); see module docstring
    xT_rev = np.ascontiguousarray(x.T[::-1, :]).astype(NP_BF16)  # (N, B)
    xt_t = np.ascontiguousarray(
        xT_rev.reshape(NCB, 128, B).transpose(1, 0, 2)
    )  # [p, k, b]

    # VtD[c, t] = V[t % N, c] for t in [0, 2N): doubled transpose for wrap-free
    # band extraction. band_m[c, j] = V[(r0 + j - c) % N, c] = VtD[c, N+r0+j-c]
    Vt = np.ascontiguousarray(V.T).astype(NP_BF16)
    VtD = np.concatenate([Vt, Vt], axis=1)  # (N, 2N) bf16
    flat = VtD.reshape(-1)
    isz = flat.itemsize

    in_maps = []
    for m in range(NCORES):
        r0 = m * RW
        start = N + r0  # element offset of band_m[0, 0] in flat
        band_m = np.lib.stride_tricks.as_strided(
            flat[start:], shape=(N, RW), strides=((2 * N - 1) * isz, isz),
        )
        band_rev = np.ascontiguousarray(band_m[::-1, :])  # (N, RW) bf16
        band_t = np.ascontiguousarray(
            band_rev.reshape(NCB, 128, RW).transpose(1, 0, 2)
        )  # [p, k, j]
        am = np.roll(alpha, -r0)
        in_maps.append({
            "band_t": band_t,
            "xt_t": xt_t,
            "al32": alpha,
            "a2bf": np.concatenate([am, am]).astype(NP_BF16),
        })
    return in_maps


def gather_output(results) -> np.ndarray:
    return np.concatenate(
        [results[m]["out"].astype(np.float32) for m in range(NCORES)], axis=1
    )


def kernel(x: np.ndarray, V: np.ndarray, alpha: np.ndarray) -> np.ndarray:
    in_maps = prepare_in_maps(x, V, alpha)
    nc = _get_program()
    res = run_bass_kernel_spmd(nc, in_maps, core_ids=list(range(NCORES)))
    return gather_output(res.results)
